# revision 13
# baseline (speedup 1.0000x reference)
"""ArDCA pseudo-likelihood loss on 8 Trainium2 NeuronCores.

Math (reference): for samples X (M,L) over alphabet Q with weights W,
    pair[m,i,a] = sum_{j<i} J[i,j,a,X[m,j]]
    logits = h_pos + pair ;  loss = -sum_{m,i} W[m]*log_softmax(logits)[gold]
                              + lam_h*|h|^2 + lam_j*|tril(J)|^2

Strategy: data-parallel over M (1024 samples/core).  The one-hot einsum is a
dense TensorEngine matmul: out[m, (i,a)] += onehotT[(j,b), m].T @ J[(j,b),(i,a)]
with K = L*Q = 5376 contraction packed 6 j-positions per 128-row K-tile and the
strict lower-triangle (j<i) skipped at tile granularity.  J (tril-masked,
scaled by 64, fp8-e4m3, column-packed) stays SBUF-resident; h_pos rides in as a
bias row of K-tile 0 against an all-ones row of the one-hot.  Epilogue per
(m-tile, i-block): exp on ScalarE (scale=1/64 folded in), 21-wide segment
reduce + ln (logZ), gold logit via tensor_tensor_reduce against the m-oriented
one-hot.  regJ/regH via ScalarE Square over the resident J.  Each core emits a
(128,1) partial column; the host sums 8*128 values into the scalar loss.
"""

import os
import sys

import numpy as np
import ml_dtypes

try:
    import concourse.bass as bass  # noqa: F401
except ImportError:  # pragma: no cover
    sys.path.insert(0, "/opt/trn_rl_repo")

import concourse.bass as bass
import concourse.mybir as mybir
import concourse.tile as tile
from concourse import bacc
from concourse.bass_utils import run_bass_kernel_spmd

# ---------------------------------------------------------------- constants
M, L, Q = 8192, 256, 21
LAMBDA_H = 1e-06
LAMBDA_J = 0.0001

NCORES = 8
MC = M // NCORES        # 1024 samples per core
MT = MC // 128          # 8 m-tiles per core
LQ = L * Q              # 5376

JPK = 6                 # j-positions per K-tile (6*21=126 <= 128)
KT = (L + JPK - 1) // JPK   # 43 K-tiles
IB = 24                 # i-positions per i-block (24*21=504 <= 512 psum bank)
NIB = (L + IB - 1) // IB    # 11 i-blocks (10 of 24 + 1 of 16)
IB_N = [min(IB, L - IB * b) * Q for b in range(NIB)]  # 504 .. 336
SCALE = 64.0            # fp8 pre-scale on J / h

FP8 = ml_dtypes.float8_e4m3
BF16 = ml_dtypes.bfloat16

# first i-block each K-tile contributes to: need some i in block with i > 6*kt
BMIN = [(JPK * kt + 1) // IB for kt in range(KT)]
# last K-tile contributing to i-block b:  j <= i_max-1 = min(IB*(b+1),L)-2
LASTKT = [min(KT - 1, (IB * (b + 1) - 2) // JPK) for b in range(NIB)]
# packed J column widths / offsets (per partition, in elements)
JW = [LQ - 504 * BMIN[kt] for kt in range(KT)]
JOFS = np.concatenate([[0], np.cumsum(JW)]).astype(int)
TOTW = int(JOFS[-1])

WAVES = [(0, 8), (8, NIB)]  # i-block ranges sharing the 8 psum banks

_DT = mybir.dt


# ---------------------------------------------------------------- host prep
def _prep_shared(J, h_pos):
    """tril-mask, scale, transpose J into the packed (128, TOTW) fp8 rhs."""
    J = np.asarray(J, np.float32)
    h = np.asarray(h_pos, np.float32)
    mask = np.tril(np.ones((L, L), np.float32), k=-1)
    out = np.empty((128, TOTW), FP8)
    for kt in range(KT):
        j0 = JPK * kt
        jw = min(JPK, L - j0)
        blk = J[:, j0 : j0 + jw] * mask[:, j0 : j0 + jw, None, None]  # (i,j',a,b)
        t = blk.transpose(1, 3, 0, 2).reshape(jw * Q, LQ)  # rows=(j',b), cols=(i,a)
        tl = np.zeros((128, LQ), np.float32)
        tl[: jw * Q] = t
        if kt == 0:
            tl[126] = h.reshape(LQ)
        tl *= SCALE
        out[:, JOFS[kt] : JOFS[kt + 1]] = tl[:, 504 * BMIN[kt] :].astype(FP8)
    return out


def _prep_core(Xs, Ws):
    """Per-core one-hot (both orientations) + weight tile."""
    Xs = np.asarray(Xs)
    jj = np.arange(L)
    # K-oriented one-hot: (KT, 128, MC); row = 21*(j%6)+b, col = m
    xoht = np.zeros((KT, 128, MC), np.float32)
    rows = Q * (jj % JPK)[None, :] + Xs  # (MC, L)
    kts = (jj // JPK)[None, :].repeat(MC, 0)  # (MC, L)
    mm = np.arange(MC)[:, None].repeat(L, 1)
    xoht[kts.ravel(), rows.ravel(), mm.ravel()] = 1.0
    xoht[0, 126, :] = 1.0  # bias row pairs with h row in J
    # m-oriented one-hot: (MT, 128, LQ)
    ohm = np.zeros((MC, LQ), np.float32)
    ohm[np.arange(MC)[:, None], Q * jj[None, :] + Xs] = 1.0
    wt = np.ascontiguousarray(np.asarray(Ws, np.float32).reshape(MT, 128).T)
    return (
        xoht.astype(FP8),
        ohm.reshape(MT, 128, LQ).astype(BF16),
        wt,
    )


# ---------------------------------------------------------------- device code
def _build_graph():
    nc = bacc.Bacc(
        "TRN2", target_bir_lowering=False, debug=False, num_devices=NCORES
    )
    jd = nc.dram_tensor("jrs", [128, TOTW], _DT.float8e4, kind="ExternalInput")
    xd = nc.dram_tensor("xoht", [KT, 128, MC], _DT.float8e4, kind="ExternalInput")
    od = nc.dram_tensor("ohm", [MT, 128, LQ], _DT.bfloat16, kind="ExternalInput")
    wd = nc.dram_tensor("wt", [128, MT], _DT.float32, kind="ExternalInput")
    hd = nc.dram_tensor("hrow", [128, LQ // 128], _DT.float32, kind="ExternalInput")
    outd = nc.dram_tensor("out", [128, 1], _DT.float32, kind="ExternalOutput")

    f32, fp8, bf16 = _DT.float32, _DT.float8e4, _DT.bfloat16
    NCOL = MT * NIB  # 88 logZ / gold accumulator columns

    with tile.TileContext(nc) as tc:
        with (
            tc.tile_pool(name="jres", bufs=1) as jpool,
            tc.tile_pool(name="xres", bufs=1) as xpool,
            tc.tile_pool(name="consts", bufs=1) as cpool,
            tc.tile_pool(name="psum", bufs=8, space="PSUM") as ppool,
            tc.tile_pool(name="exps", bufs=3) as epool,
            tc.tile_pool(name="prods", bufs=2) as prpool,
            tc.tile_pool(name="ohms", bufs=3) as opool,
            tc.tile_pool(name="small", bufs=4) as spool,
            tc.tile_pool(name="sq", bufs=1) as sqpool,
        ):
            jt = jpool.tile([128, TOTW], fp8)
            for kt in range(KT):
                nc.sync.dma_start(
                    jt[:, JOFS[kt] : JOFS[kt + 1]], jd[:, JOFS[kt] : JOFS[kt + 1]]
                )
            xo = xpool.tile([128, KT * MC], fp8)
            for kt in range(KT):
                nc.sync.dma_start(xo[:, kt * MC : (kt + 1) * MC], xd[kt])
            wt = cpool.tile([128, MT], f32)
            nc.sync.dma_start(wt[:], wd[:])

            zbuf = cpool.tile([128, NCOL], f32)
            gbuf = cpool.tile([128, NCOL], f32)
            rjcols = cpool.tile([128, KT], f32)
            nc.vector.memset(rjcols[:], 0.0)
            rhcol = cpool.tile([128, 1], f32)
            nc.vector.memset(rhcol[:], 0.0)

            sq = sqpool.tile([128, LQ], bf16)

            def epilogue(t, ib, ps):
                w = IB_N[ib]
                nI = w // Q
                idx = t * NIB + ib
                e = epool.tile([128, 504], f32, tag="exp")
                nc.scalar.activation(
                    e[:, :w], ps[:, :w], mybir.ActivationFunctionType.Exp,
                    scale=1.0 / SCALE,
                )
                s = spool.tile([128, IB], f32, tag="s")
                nc.vector.reduce_sum(
                    s[:, :nI],
                    e[:, :w].rearrange("p (i a) -> p i a", a=Q),
                    axis=mybir.AxisListType.X,
                )
                ln = spool.tile([128, IB], f32, tag="ln")
                nc.scalar.activation(
                    ln[:, :nI], s[:, :nI], mybir.ActivationFunctionType.Ln,
                    accum_out=zbuf[:, idx : idx + 1],
                )
                oh = opool.tile([128, 504], bf16, tag="oh")
                nc.sync.dma_start(oh[:, :w], od[t, :, 504 * ib : 504 * ib + w])
                pr = prpool.tile([128, 504], f32, tag="pr")
                nc.vector.tensor_tensor(
                    out=pr[:, :w], in0=ps[:, :w], in1=oh[:, :w],
                    op=mybir.AluOpType.mult,
                )
                nc.vector.reduce_sum(
                    gbuf[:, idx : idx + 1], pr[:, :w], axis=mybir.AxisListType.X
                )

            # interleave the regJ squares across the m-tile loop to fill ACT gaps
            sq_sched = {t: [] for t in range(MT)}
            for kt in range(KT):
                sq_sched[min(kt * MT // KT, MT - 1)].append(kt)

            for t in range(MT):
                for ib_lo, ib_hi in WAVES:
                    psums = {}
                    for ib in range(ib_lo, ib_hi):
                        psums[ib] = ppool.tile([128, 504], f32, tag="ps", name=f"ps_{t}_{ib}")
                    kt_hi = max(LASTKT[ib] for ib in range(ib_lo, ib_hi)) + 1
                    for kt in range(kt_hi):
                        lhs = xo[:, kt * MC + t * 128 : kt * MC + (t + 1) * 128]
                        for ib in range(max(ib_lo, BMIN[kt]), ib_hi):
                            if kt > LASTKT[ib]:
                                continue
                            w = IB_N[ib]
                            c0 = JOFS[kt] + 504 * (ib - BMIN[kt])
                            nc.tensor.matmul(
                                psums[ib][:, :w],
                                lhs,
                                jt[:, c0 : c0 + w],
                                start=(kt == 0),
                                stop=(kt == LASTKT[ib]),
                            )
                    for ib in range(ib_lo, ib_hi):
                        epilogue(t, ib, psums[ib])
                for kt in sq_sched[t]:
                    nc.scalar.activation(
                        sq[:126, : JW[kt]],
                        jt[:126, JOFS[kt] : JOFS[kt + 1]],
                        mybir.ActivationFunctionType.Square,
                        accum_out=rjcols[:126, kt : kt + 1],
                    )

            # regH from a separate unscaled copy of h, reshaped (128, 42)
            hrow = cpool.tile([128, LQ // 128], f32)
            nc.sync.dma_start(hrow[:], hd[:])
            nc.scalar.activation(
                sq[:, : LQ // 128],
                hrow[:],
                mybir.ActivationFunctionType.Square,
                accum_out=rhcol[:],
            )

            # final combine: per-partition partial of the loss
            dbuf = cpool.tile([128, NCOL], f32)
            nc.vector.tensor_scalar(
                out=dbuf[:],
                in0=gbuf[:],
                scalar1=-1.0 / SCALE,
                scalar2=None,
                op0=mybir.AluOpType.mult,
            )
            nc.vector.tensor_tensor(
                out=dbuf[:], in0=dbuf[:], in1=zbuf[:], op=mybir.AluOpType.add
            )
            dm = spool.tile([128, MT], f32, tag="dm")
            nc.vector.reduce_sum(
                dm[:],
                dbuf[:].rearrange("p (t i) -> p t i", i=NIB),
                axis=mybir.AxisListType.X,
            )
            nll = spool.tile([128, 1], f32, tag="nll")
            wprod = spool.tile([128, MT], f32, tag="wprod")
            nc.vector.tensor_tensor(
                out=wprod[:], in0=dm[:], in1=wt[:], op=mybir.AluOpType.mult
            )
            nc.vector.reduce_sum(nll[:], wprod[:], axis=mybir.AxisListType.X)
            rj = spool.tile([128, 1], f32, tag="rj")
            nc.vector.reduce_sum(rj[:], rjcols[:], axis=mybir.AxisListType.X)
            nc.vector.tensor_scalar(
                out=rj[:], in0=rj[:],
                scalar1=LAMBDA_J / (SCALE * SCALE), scalar2=None,
                op0=mybir.AluOpType.mult,
            )
            nc.vector.tensor_scalar(
                out=rhcol[:], in0=rhcol[:],
                scalar1=LAMBDA_H / NCORES, scalar2=None,
                op0=mybir.AluOpType.mult,
            )
            ocol = spool.tile([128, 1], f32, tag="ocol")
            nc.vector.tensor_tensor(
                out=ocol[:], in0=nll[:], in1=rj[:], op=mybir.AluOpType.add
            )
            nc.vector.tensor_tensor(
                out=ocol[:], in0=ocol[:], in1=rhcol[:], op=mybir.AluOpType.add
            )
            nc.sync.dma_start(outd[:], ocol[:])

    nc.compile()
    return nc


_GRAPH = None


def _graph():
    global _GRAPH
    if _GRAPH is None:
        _GRAPH = _build_graph()
    return _GRAPH


# ------------------------------------------------------- persistent runner
# Mirrors concourse.bass2jax.run_bass_via_pjrt but caches the jitted
# shard_map executable so repeated calls don't re-trace/re-compile.
class _Runner:
    def __init__(self, nc):
        import jax
        from jax.sharding import Mesh, PartitionSpec
        from jax.experimental.shard_map import shard_map
        import concourse.mybir as mybir
        from concourse import bass2jax

        bass2jax.install_neuronx_cc_hook()
        partition_name = (
            nc.partition_id_tensor.name if nc.partition_id_tensor else None
        )
        in_names, out_names, out_avals, zero_outs = [], [], [], []
        for alloc in nc.m.functions[0].allocations:
            if not isinstance(alloc, mybir.MemoryLocationSet):
                continue
            name = alloc.memorylocations[0].name
            if alloc.kind == "ExternalInput":
                if name != partition_name:
                    in_names.append(name)
            elif alloc.kind == "ExternalOutput":
                shape = tuple(alloc.tensor_shape)
                dtype = mybir.dt.np(alloc.dtype)
                out_names.append(name)
                out_avals.append(jax.core.ShapedArray(shape, dtype))
                zero_outs.append(np.zeros(shape, dtype))
        n_params = len(in_names)
        all_names = in_names + out_names
        if partition_name is not None:
            all_names = all_names + [partition_name]

        def _body(*args):
            operands = list(args)
            if partition_name is not None:
                operands.append(bass2jax.partition_id_tensor())
            outs = bass2jax._bass_exec_p.bind(
                *operands,
                out_avals=tuple(out_avals),
                in_names=tuple(all_names),
                out_names=tuple(out_names),
                lowering_input_output_aliases=(),
                sim_require_finite=True,
                sim_require_nnan=True,
                nc=nc,
            )
            return tuple(outs)

        devices = jax.devices()[:NCORES]
        mesh = Mesh(np.asarray(devices), ("core",))
        self.mesh = mesh
        nin = n_params + len(out_names)
        self._jit = jax.jit(
            shard_map(
                _body,
                mesh=mesh,
                in_specs=(PartitionSpec("core"),) * nin,
                out_specs=(PartitionSpec("core"),) * len(out_names),
                check_rep=False,
            ),
            keep_unused=True,
        )
        self.in_names = in_names
        self.out_names = out_names
        self.out_avals = out_avals
        self.zero_outs = zero_outs
        self._jax = jax

    def put_inputs(self, in_maps, device_resident=True):
        """Concatenate per-core inputs and return the arg list."""
        concat = [
            np.concatenate(
                [np.asarray(in_maps[c][n]) for c in range(NCORES)], axis=0
            )
            for n in self.in_names
        ]
        zeros = [
            np.zeros((NCORES * z.shape[0], *z.shape[1:]), z.dtype)
            for z in self.zero_outs
        ]
        args = concat + zeros
        if device_resident:
            from jax.sharding import NamedSharding, PartitionSpec

            sh = NamedSharding(self.mesh, PartitionSpec("core"))
            args = [self._jax.device_put(a, sh) for a in args]
            self._jax.block_until_ready(args)
        return args

    def run(self, args):
        outs = self._jit(*args)
        self._jax.block_until_ready(outs)
        return {
            n: np.asarray(outs[i]).reshape(NCORES, *self.out_avals[i].shape)
            for i, n in enumerate(self.out_names)
        }


_RUNNER = None


def _runner():
    global _RUNNER
    if _RUNNER is None:
        _RUNNER = _Runner(_graph())
    return _RUNNER


def _make_in_maps(X_idx, W, h_pos, J):
    X_idx = np.asarray(X_idx)
    W = np.asarray(W, np.float32)
    jrs = _prep_shared(J, h_pos)
    hrow = np.ascontiguousarray(
        np.asarray(h_pos, np.float32).reshape(128, LQ // 128)
    )
    in_maps = []
    for c in range(NCORES):
        xoht, ohm, wt = _prep_core(
            X_idx[c * MC : (c + 1) * MC], W[c * MC : (c + 1) * MC]
        )
        in_maps.append(
            {"jrs": jrs, "xoht": xoht, "ohm": ohm, "wt": wt, "hrow": hrow}
        )
    return in_maps


# ---------------------------------------------------------------- entry point
def kernel(X_idx, W, h_pos, J):
    r = _runner()
    out = r.run(r.put_inputs(_make_in_maps(X_idx, W, h_pos, J)))
    return np.float32(np.asarray(out["out"], np.float64).sum())


def bench(X_idx, W, h_pos, J, reps=20):
    """Return (loss, mean_exec_seconds) amortized over reps (incl. RPC)."""
    import time

    r = _runner()
    args = r.put_inputs(_make_in_maps(X_idx, W, h_pos, J))
    out = r.run(args)  # warm-up / compile
    t0 = time.time()
    for _ in range(reps):
        out = r.run(args)
    dt = (time.time() - t0) / reps
    return np.float32(np.asarray(out["out"], np.float64).sum()), dt


# revision 15
# speedup vs baseline: 1.1288x; 1.1288x over previous
"""ArDCA pseudo-likelihood loss on 8 Trainium2 NeuronCores.

Math (reference): for samples X (M,L) over alphabet Q with weights W,
    pair[m,i,a] = sum_{j<i} J[i,j,a,X[m,j]]
    logits = h_pos + pair ;  loss = -sum_{m,i} W[m]*log_softmax(logits)[gold]
                              + lam_h*|h|^2 + lam_j*|tril(J)|^2

Strategy: data-parallel over M (1024 samples/core).  The one-hot einsum is a
dense TensorEngine matmul: out[m, (i,a)] += onehotT[(j,b), m].T @ J[(j,b),(i,a)]
with K = L*Q = 5376 contraction packed 6 j-positions per 128-row K-tile and the
strict lower-triangle (j<i) skipped at tile granularity.  J (tril-masked,
scaled by 64, fp8-e4m3, column-packed) stays SBUF-resident; h_pos rides in as a
bias row of K-tile 0 against an all-ones row of the one-hot.  Epilogue per
(m-tile, i-block): exp on ScalarE (scale=1/64 folded in), 21-wide segment
reduce + ln (logZ), gold logit via tensor_tensor_reduce against the m-oriented
one-hot.  regJ/regH via ScalarE Square over the resident J.  Each core emits a
(128,1) partial column; the host sums 8*128 values into the scalar loss.
"""

import os
import sys

import numpy as np
import ml_dtypes

try:
    import concourse.bass as bass  # noqa: F401
except ImportError:  # pragma: no cover
    sys.path.insert(0, "/opt/trn_rl_repo")

import concourse.bass as bass
import concourse.mybir as mybir
import concourse.tile as tile
from concourse import bacc
from concourse.bass_utils import run_bass_kernel_spmd

# ---------------------------------------------------------------- constants
M, L, Q = 8192, 256, 21
LAMBDA_H = 1e-06
LAMBDA_J = 0.0001

NCORES = 8
MC = M // NCORES        # 1024 samples per core
MT = MC // 128          # 8 m-tiles per core
LQ = L * Q              # 5376

JPK = 6                 # j-positions per K-tile (6*21=126 <= 128)
KT = (L + JPK - 1) // JPK   # 43 K-tiles
IB = 24                 # i-positions per i-block (24*21=504 <= 512 psum bank)
NIB = (L + IB - 1) // IB    # 11 i-blocks (10 of 24 + 1 of 16)
IB_N = [min(IB, L - IB * b) * Q for b in range(NIB)]  # 504 .. 336
SCALE = 64.0            # fp8 pre-scale on J / h

FP8 = ml_dtypes.float8_e4m3
BF16 = ml_dtypes.bfloat16

# first i-block each K-tile contributes to: need some i in block with i > 6*kt
BMIN = [(JPK * kt + 1) // IB for kt in range(KT)]
# last K-tile contributing to i-block b:  j <= i_max-1 = min(IB*(b+1),L)-2
LASTKT = [min(KT - 1, (IB * (b + 1) - 2) // JPK) for b in range(NIB)]
# packed J column widths / offsets (per partition, in elements)
JW = [LQ - 504 * BMIN[kt] for kt in range(KT)]
JOFS = np.concatenate([[0], np.cumsum(JW)]).astype(int)
TOTW = int(JOFS[-1])

WAVES = [(0, 8), (8, NIB)]  # i-block ranges sharing the 8 psum banks

_DT = mybir.dt


# ---------------------------------------------------------------- host prep
def _prep_shared(J, h_pos):
    """tril-mask, scale, transpose J into the packed (128, TOTW) fp8 rhs."""
    J = np.asarray(J, np.float32)
    h = np.asarray(h_pos, np.float32)
    mask = np.tril(np.ones((L, L), np.float32), k=-1)
    out = np.empty((128, TOTW), FP8)
    for kt in range(KT):
        j0 = JPK * kt
        jw = min(JPK, L - j0)
        blk = J[:, j0 : j0 + jw] * mask[:, j0 : j0 + jw, None, None]  # (i,j',a,b)
        t = blk.transpose(1, 3, 0, 2).reshape(jw * Q, LQ)  # rows=(j',b), cols=(i,a)
        tl = np.zeros((128, LQ), np.float32)
        tl[: jw * Q] = t
        if kt == 0:
            tl[126] = h.reshape(LQ)
        tl *= SCALE
        out[:, JOFS[kt] : JOFS[kt + 1]] = tl[:, 504 * BMIN[kt] :].astype(FP8)
    return out


def _prep_core(Xs, Ws):
    """Per-core one-hot (both orientations) + weight tile."""
    Xs = np.asarray(Xs)
    jj = np.arange(L)
    # K-oriented one-hot: (KT, 128, MC); row = 21*(j%6)+b, col = m
    xoht = np.zeros((KT, 128, MC), np.float32)
    rows = Q * (jj % JPK)[None, :] + Xs  # (MC, L)
    kts = (jj // JPK)[None, :].repeat(MC, 0)  # (MC, L)
    mm = np.arange(MC)[:, None].repeat(L, 1)
    xoht[kts.ravel(), rows.ravel(), mm.ravel()] = 1.0
    xoht[0, 126, :] = 1.0  # bias row pairs with h row in J
    # m-oriented one-hot: (MT, 128, LQ)
    ohm = np.zeros((MC, LQ), np.float32)
    ohm[np.arange(MC)[:, None], Q * jj[None, :] + Xs] = 1.0
    wt = np.ascontiguousarray(np.asarray(Ws, np.float32).reshape(MT, 128).T)
    return (
        xoht.astype(FP8),
        ohm.reshape(MT, 128, LQ).astype(BF16),
        wt,
    )


# ---------------------------------------------------------------- device code
def _build_graph():
    nc = bacc.Bacc(
        "TRN2", target_bir_lowering=False, debug=False, num_devices=NCORES
    )
    jd = nc.dram_tensor("jrs", [128, TOTW], _DT.float8e4, kind="ExternalInput")
    xd = nc.dram_tensor("xoht", [KT, 128, MC], _DT.float8e4, kind="ExternalInput")
    od = nc.dram_tensor("ohm", [MT, 128, LQ], _DT.bfloat16, kind="ExternalInput")
    wd = nc.dram_tensor("wt", [128, MT], _DT.float32, kind="ExternalInput")
    hd = nc.dram_tensor("hrow", [128, LQ // 128], _DT.float32, kind="ExternalInput")
    outd = nc.dram_tensor("out", [128, 1], _DT.float32, kind="ExternalOutput")

    f32, fp8, bf16 = _DT.float32, _DT.float8e4, _DT.bfloat16
    NCOL = MT * NIB  # 88 logZ / gold accumulator columns

    with tile.TileContext(nc) as tc:
        with (
            tc.tile_pool(name="jres", bufs=1) as jpool,
            tc.tile_pool(name="xres", bufs=1) as xpool,
            tc.tile_pool(name="consts", bufs=1) as cpool,
            tc.tile_pool(name="psum", bufs=8, space="PSUM") as ppool,
            tc.tile_pool(name="exps", bufs=3) as epool,
            tc.tile_pool(name="prods", bufs=2) as prpool,
            tc.tile_pool(name="ohms", bufs=3) as opool,
            tc.tile_pool(name="small", bufs=4) as spool,
            tc.tile_pool(name="sq", bufs=1) as sqpool,
        ):
            jt = jpool.tile([128, TOTW], fp8)
            xo = xpool.tile([128, KT * MC], fp8)
            # interleave per-kt so m-tile 0's first matmuls unblock early
            for kt in range(KT):
                nc.sync.dma_start(xo[:, kt * MC : (kt + 1) * MC], xd[kt])
                nc.sync.dma_start(
                    jt[:, JOFS[kt] : JOFS[kt + 1]], jd[:, JOFS[kt] : JOFS[kt + 1]]
                )
            wt = cpool.tile([128, MT], f32)
            nc.sync.dma_start(wt[:], wd[:])

            # S[m, t*L + i] = sum_a exp(logits[m,i,a]/1) for m-tile t
            sbig = cpool.tile([128, MT * L], f32)
            gbuf = cpool.tile([128, NCOL], f32)
            rjcols = cpool.tile([128, KT], f32)
            nc.vector.memset(rjcols[:], 0.0)
            rhcol = cpool.tile([128, 1], f32)
            nc.vector.memset(rhcol[:], 0.0)

            sq = sqpool.tile([128, LQ], bf16)

            def epilogue(t, ib, ps):
                w = IB_N[ib]
                nI = w // Q
                idx = t * NIB + ib
                e = epool.tile([128, 504], f32, tag="exp")
                nc.scalar.activation(
                    e[:, :w], ps[:, :w], mybir.ActivationFunctionType.Exp,
                    scale=1.0 / SCALE,
                )
                c0 = t * L + IB * ib
                nc.vector.reduce_sum(
                    sbig[:, c0 : c0 + nI],
                    e[:, :w].rearrange("p (i a) -> p i a", a=Q),
                    axis=mybir.AxisListType.X,
                )
                oh = opool.tile([128, 504], bf16, tag="oh")
                nc.sync.dma_start(oh[:, :w], od[t, :, 504 * ib : 504 * ib + w])
                pr = prpool.tile([128, 504], f32, tag="pr")
                nc.vector.tensor_tensor(
                    out=pr[:, :w], in0=ps[:, :w], in1=oh[:, :w],
                    op=mybir.AluOpType.mult,
                )
                nc.vector.reduce_sum(
                    gbuf[:, idx : idx + 1], pr[:, :w], axis=mybir.AxisListType.X
                )

            # interleave the regJ squares across the m-tile loop to fill ACT gaps
            sq_sched = {t: [] for t in range(MT)}
            for kt in range(KT):
                sq_sched[min(kt * MT // KT, MT - 1)].append(kt)

            for t in range(MT):
                for ib_lo, ib_hi in WAVES:
                    psums = {}
                    for ib in range(ib_lo, ib_hi):
                        psums[ib] = ppool.tile([128, 504], f32, tag="ps", name=f"ps_{t}_{ib}")
                    kt_hi = max(LASTKT[ib] for ib in range(ib_lo, ib_hi)) + 1
                    for kt in range(kt_hi):
                        lhs = xo[:, kt * MC + t * 128 : kt * MC + (t + 1) * 128]
                        for ib in range(max(ib_lo, BMIN[kt]), ib_hi):
                            if kt > LASTKT[ib]:
                                continue
                            w = IB_N[ib]
                            c0 = JOFS[kt] + 504 * (ib - BMIN[kt])
                            nc.tensor.matmul(
                                psums[ib][:, :w],
                                lhs,
                                jt[:, c0 : c0 + w],
                                start=(kt == 0),
                                stop=(kt == LASTKT[ib]),
                            )
                    for ib in range(ib_lo, ib_hi):
                        epilogue(t, ib, psums[ib])
                for kt in sq_sched[t]:
                    nc.scalar.activation(
                        sq[:126, : JW[kt]],
                        jt[:126, JOFS[kt] : JOFS[kt + 1]],
                        mybir.ActivationFunctionType.Square,
                        accum_out=rjcols[:126, kt : kt + 1],
                    )

            # regH from a separate unscaled copy of h, reshaped (128, 42)
            hrow = cpool.tile([128, LQ // 128], f32)
            nc.sync.dma_start(hrow[:], hd[:])
            nc.scalar.activation(
                sq[:, : LQ // 128],
                hrow[:],
                mybir.ActivationFunctionType.Square,
                accum_out=rhcol[:],
            )

            # batched Ln pass: one op per m-tile, single table-set load
            zcols = cpool.tile([128, MT], f32)
            lns = cpool.tile([128, L], f32)
            for t in range(MT):
                nc.scalar.activation(
                    lns[:], sbig[:, t * L : (t + 1) * L],
                    mybir.ActivationFunctionType.Ln,
                    accum_out=zcols[:, t : t + 1],
                )

            # final combine: per-partition partial of the loss
            dm = spool.tile([128, MT], f32, tag="dm")
            nc.vector.reduce_sum(
                dm[:],
                gbuf[:].rearrange("p (t i) -> p t i", i=NIB),
                axis=mybir.AxisListType.X,
            )
            nc.vector.tensor_scalar(
                out=dm[:], in0=dm[:],
                scalar1=-1.0 / SCALE, scalar2=None,
                op0=mybir.AluOpType.mult,
            )
            nc.vector.tensor_tensor(
                out=dm[:], in0=dm[:], in1=zcols[:], op=mybir.AluOpType.add
            )
            nll = spool.tile([128, 1], f32, tag="nll")
            wprod = spool.tile([128, MT], f32, tag="wprod")
            nc.vector.tensor_tensor(
                out=wprod[:], in0=dm[:], in1=wt[:], op=mybir.AluOpType.mult
            )
            nc.vector.reduce_sum(nll[:], wprod[:], axis=mybir.AxisListType.X)
            rj = spool.tile([128, 1], f32, tag="rj")
            nc.vector.reduce_sum(rj[:], rjcols[:], axis=mybir.AxisListType.X)
            nc.vector.tensor_scalar(
                out=rj[:], in0=rj[:],
                scalar1=LAMBDA_J / (SCALE * SCALE), scalar2=None,
                op0=mybir.AluOpType.mult,
            )
            nc.vector.tensor_scalar(
                out=rhcol[:], in0=rhcol[:],
                scalar1=LAMBDA_H / NCORES, scalar2=None,
                op0=mybir.AluOpType.mult,
            )
            ocol = spool.tile([128, 1], f32, tag="ocol")
            nc.vector.tensor_tensor(
                out=ocol[:], in0=nll[:], in1=rj[:], op=mybir.AluOpType.add
            )
            nc.vector.tensor_tensor(
                out=ocol[:], in0=ocol[:], in1=rhcol[:], op=mybir.AluOpType.add
            )
            nc.sync.dma_start(outd[:], ocol[:])

    nc.compile()
    return nc


_GRAPH = None


def _graph():
    global _GRAPH
    if _GRAPH is None:
        _GRAPH = _build_graph()
    return _GRAPH


# ------------------------------------------------------- persistent runner
# Mirrors concourse.bass2jax.run_bass_via_pjrt but caches the jitted
# shard_map executable so repeated calls don't re-trace/re-compile.
class _Runner:
    def __init__(self, nc):
        import jax
        from jax.sharding import Mesh, PartitionSpec
        from jax.experimental.shard_map import shard_map
        import concourse.mybir as mybir
        from concourse import bass2jax

        bass2jax.install_neuronx_cc_hook()
        partition_name = (
            nc.partition_id_tensor.name if nc.partition_id_tensor else None
        )
        in_names, out_names, out_avals, zero_outs = [], [], [], []
        for alloc in nc.m.functions[0].allocations:
            if not isinstance(alloc, mybir.MemoryLocationSet):
                continue
            name = alloc.memorylocations[0].name
            if alloc.kind == "ExternalInput":
                if name != partition_name:
                    in_names.append(name)
            elif alloc.kind == "ExternalOutput":
                shape = tuple(alloc.tensor_shape)
                dtype = mybir.dt.np(alloc.dtype)
                out_names.append(name)
                out_avals.append(jax.core.ShapedArray(shape, dtype))
                zero_outs.append(np.zeros(shape, dtype))
        n_params = len(in_names)
        all_names = in_names + out_names
        if partition_name is not None:
            all_names = all_names + [partition_name]

        def _body(*args):
            operands = list(args)
            if partition_name is not None:
                operands.append(bass2jax.partition_id_tensor())
            outs = bass2jax._bass_exec_p.bind(
                *operands,
                out_avals=tuple(out_avals),
                in_names=tuple(all_names),
                out_names=tuple(out_names),
                lowering_input_output_aliases=(),
                sim_require_finite=True,
                sim_require_nnan=True,
                nc=nc,
            )
            return tuple(outs)

        devices = jax.devices()[:NCORES]
        mesh = Mesh(np.asarray(devices), ("core",))
        self.mesh = mesh
        nin = n_params + len(out_names)
        self._jit = jax.jit(
            shard_map(
                _body,
                mesh=mesh,
                in_specs=(PartitionSpec("core"),) * nin,
                out_specs=(PartitionSpec("core"),) * len(out_names),
                check_rep=False,
            ),
            keep_unused=True,
        )
        self.in_names = in_names
        self.out_names = out_names
        self.out_avals = out_avals
        self.zero_outs = zero_outs
        self._jax = jax

    def put_inputs(self, in_maps, device_resident=True):
        """Concatenate per-core inputs and return the arg list."""
        concat = [
            np.concatenate(
                [np.asarray(in_maps[c][n]) for c in range(NCORES)], axis=0
            )
            for n in self.in_names
        ]
        zeros = [
            np.zeros((NCORES * z.shape[0], *z.shape[1:]), z.dtype)
            for z in self.zero_outs
        ]
        args = concat + zeros
        if device_resident:
            from jax.sharding import NamedSharding, PartitionSpec

            sh = NamedSharding(self.mesh, PartitionSpec("core"))
            args = [self._jax.device_put(a, sh) for a in args]
            self._jax.block_until_ready(args)
        return args

    def run(self, args):
        outs = self._jit(*args)
        self._jax.block_until_ready(outs)
        return {
            n: np.asarray(outs[i]).reshape(NCORES, *self.out_avals[i].shape)
            for i, n in enumerate(self.out_names)
        }


_RUNNER = None


def _runner():
    global _RUNNER
    if _RUNNER is None:
        _RUNNER = _Runner(_graph())
    return _RUNNER


def _make_in_maps(X_idx, W, h_pos, J):
    X_idx = np.asarray(X_idx)
    W = np.asarray(W, np.float32)
    jrs = _prep_shared(J, h_pos)
    hrow = np.ascontiguousarray(
        np.asarray(h_pos, np.float32).reshape(128, LQ // 128)
    )
    in_maps = []
    for c in range(NCORES):
        xoht, ohm, wt = _prep_core(
            X_idx[c * MC : (c + 1) * MC], W[c * MC : (c + 1) * MC]
        )
        in_maps.append(
            {"jrs": jrs, "xoht": xoht, "ohm": ohm, "wt": wt, "hrow": hrow}
        )
    return in_maps


# ---------------------------------------------------------------- entry point
def kernel(X_idx, W, h_pos, J):
    r = _runner()
    out = r.run(r.put_inputs(_make_in_maps(X_idx, W, h_pos, J)))
    return np.float32(np.asarray(out["out"], np.float64).sum())


def bench(X_idx, W, h_pos, J, reps=20):
    """Return (loss, mean_exec_seconds) amortized over reps (incl. RPC)."""
    import time

    r = _runner()
    args = r.put_inputs(_make_in_maps(X_idx, W, h_pos, J))
    out = r.run(args)  # warm-up / compile
    t0 = time.time()
    for _ in range(reps):
        out = r.run(args)
    dt = (time.time() - t0) / reps
    return np.float32(np.asarray(out["out"], np.float64).sum()), dt
